# revision 15
# baseline (speedup 1.0000x reference)
"""BioJepa dense transformer on 8 TRN2 NeuronCores — v2.

Sharding: data-parallel over batch (B=8 -> 1 batch element per core).

Design vs v1 baseline: x residual stays token-major f32 in SBUF
([128, 24, 768]); per LN phase, ln(x)^T is materialized ONCE as a
feature-major bf16 tensor xT [128, 6, 3072] (PE transposes, quad-packed
drains). All projections then run as dense back-to-back matmul streams
(LDWEIGHTS overlaps, PE stays at full clock):
  - q projections + zden + output projections run feature-major with
    stationary weight tiles (all N=512 matmuls).
  - k/v/c/mlp-w2/final projections run token-major with stationary
    xT/y/h tiles (the feature-major tensors ARE the lhsT layout).
  - kvm accumulates across all 24 token tiles directly in 2 held PSUM
    banks (3 head-pairs packed per bank), ksum via fused ones-column.
  - z (and a-attn's alpha) computed from 12-row PE matmuls against a
    block-diagonal ksum lhsT; z broadcast to 128 partitions via a tiny
    K=2 selection matmul instead of gpsimd broadcasts.
Engine split: DVE handles PSUM drains/elu/residual/bn_stats; ACT does
exp/gelu/copies; gpsimd (Pool) does the SBUF-side LN applies.

Self-contained: hardcodes all shapes; host side shards/gathers.
"""
import numpy as np

import concourse.bass as bass
import concourse.bacc as bacc
import concourse.mybir as mybir
import concourse.tile as tile
from concourse.alu_op_type import AluOpType
from concourse.bass_utils import run_bass_kernel_spmd
from concourse.masks import make_identity

F32 = mybir.dt.float32
BF16 = mybir.dt.bfloat16
I32 = mybir.dt.int32
AF = mybir.ActivationFunctionType
OP = AluOpType

P = 128
D = 768
KD = 6          # D / 128
T = 3072
NT = 24         # T / 128
NTC = 6         # T / 512
H = 12
HD = 64
NPR = 6         # head pairs
F = 3072
NF = 24         # F / 128
L = 6
TT = 1024
CL = 2048
A_PAD = 384     # action dim 320 padded to 3*128
SPAN = 256      # mlp token span
NSP = T // SPAN

DT_MODE = 'bf16'
REPEAT = 1


def _np_dt(mdt):
    if mdt == BF16:
        import ml_dtypes
        return ml_dtypes.bfloat16
    return np.float32


def build_nc(dt_mode=DT_MODE, repeat=REPEAT, n_layers=L):
    MDT = BF16
    nc = bacc.Bacc()

    # ---- DRAM parameters ----
    x0_d = nc.declare_dram_parameter("x0", [T, D], F32, isOutput=False)
    act_d = nc.declare_dram_parameter("act", [A_PAD, 1], F32, isOutput=False)
    adw1_d = nc.declare_dram_parameter("adw1", [A_PAD, D], F32, isOutput=False)
    adw2_d = nc.declare_dram_parameter("adw2", [D, D], MDT, isOutput=False)
    aq_d = nc.declare_dram_parameter("aq", [L, D, D], MDT, isOutput=False)
    ak_d = nc.declare_dram_parameter("ak", [L, D, D], MDT, isOutput=False)
    av_d = nc.declare_dram_parameter("av", [L, D, D], MDT, isOutput=False)
    ac_d = nc.declare_dram_parameter("ac", [L, D, D], MDT, isOutput=False)
    sq_d = nc.declare_dram_parameter("sq", [L, D, D], MDT, isOutput=False)
    sk_d = nc.declare_dram_parameter("sk", [L, D, D], MDT, isOutput=False)
    sv_d = nc.declare_dram_parameter("sv", [L, D, D], MDT, isOutput=False)
    sc_d = nc.declare_dram_parameter("sc", [L, D, D], MDT, isOutput=False)
    w1_d = nc.declare_dram_parameter("w1", [L, D, F], MDT, isOutput=False)
    w2_d = nc.declare_dram_parameter("w2", [L, F, D], MDT, isOutput=False)
    wmu_d = nc.declare_dram_parameter("wmu", [D, D], MDT, isOutput=False)
    wlv_d = nc.declare_dram_parameter("wlv", [D, D], MDT, isOutput=False)
    sel_d = nc.declare_dram_parameter("sel", [H, NPR, P], MDT, isOutput=False)
    mu_d = nc.declare_dram_parameter("mu", [TT, D], F32, isOutput=True)
    lv_d = nc.declare_dram_parameter("lv", [TT, D], F32, isOutput=True)

    with tile.TileContext(nc) as tc:
        with tc.tile_pool(name="const", bufs=1) as const_p, \
             tc.tile_pool(name="xres", bufs=1) as xres_p, \
             tc.tile_pool(name="xt", bufs=1) as xt_p, \
             tc.tile_pool(name="hsb", bufs=1) as h_p, \
             tc.tile_pool(name="stat", bufs=1) as stat_p, \
             tc.tile_pool(name="wbig", bufs=2) as wbig_p, \
             tc.tile_pool(name="wmlp", bufs=2) as wmlp_p, \
             tc.tile_pool(name="fm512", bufs=2) as fm_p, \
             tc.tile_pool(name="t768", bufs=2) as t768_p, \
             tc.tile_pool(name="small", bufs=2) as small_p, \
             tc.tile_pool(name="lay", bufs=1) as lay_p, \
             tc.tile_pool(name="ps_work", bufs=2, space="PSUM") as psw_p, \
             tc.tile_pool(name="ps_half", bufs=2, space="PSUM") as psh_p, \
             tc.tile_pool(name="ps_kvm", bufs=2, space="PSUM") as psk_p:

            ident32 = const_p.tile([P, P], F32, name="ident32")
            make_identity(nc, ident32)
            identm = const_p.tile([P, P], MDT, name="identm")
            make_identity(nc, identm)

            # selection matrix for z broadcast: sel[h, pr, p] = 1 iff
            # (p<64 and h==2pr) or (p>=64 and h==2pr+1); host-provided
            sel = const_p.tile([H, NPR, P], MDT, name="sel")
            nc.sync.dma_start(out=sel, in_=sel_d[:, :, :])

            x_sb = xres_p.tile([P, NT, D], F32, name="x_sb")

            def mm(out, lhsT, rhs, start, stop, skip=False):
                nc.tensor.matmul(out, lhsT, rhs,
                                 start=start, stop=stop, skip_group_check=skip)

            def transpose128(ps_out, in_ap, ident):
                pp = in_ap.shape[0]
                b = in_ap.base_partition()
                nc.tensor.transpose(ps_out, in_ap, ident[b:b + pp, b:b + pp])

            # ---------- LN helpers ----------
            def ln_stats(x_ap, mv_out):
                """x_ap [pp, D] -> mv_out [pp, 2] (mean, var)."""
                pp = x_ap.shape[0]
                stats = small_p.tile([P, 3, 6], F32, tag="bnstats")
                xv = x_ap.rearrange("p (s c) -> p s c", s=3)
                for s in range(3):
                    nc.vector.bn_stats(out=stats[:pp, s, :], in_=xv[:, s, :])
                nc.vector.bn_aggr(out=mv_out, in_=stats[:pp])

            def newton_rsqrt(rs_out, var_ap, n_cols, pp=P):
                """rs_out [pp, n] = 1/sqrt(var_ap [pp, n] + 1e-5)."""
                vp = small_p.tile([P, NT], F32, tag="nt_vp", name="nt_vp")[:pp, :n_cols]
                nc.vector.tensor_scalar(out=vp, in0=var_ap, scalar1=1e-5,
                                        scalar2=None, op0=OP.add)
                y = rs_out
                yi = y.bitcast(I32)
                vi = vp.bitcast(I32)
                nc.vector.tensor_scalar(out=yi, in0=vi, scalar1=1,
                                        scalar2=None, op0=OP.arith_shift_right)
                nc.vector.tensor_scalar(out=yi, in0=yi, scalar1=-1,
                                        scalar2=0x5f3759df, op0=OP.mult, op1=OP.add)
                vh = small_p.tile([P, NT], F32, tag="nt_vh", name="nt_vh")[:pp, :n_cols]
                nc.vector.tensor_scalar(out=vh, in0=vp, scalar1=0.5,
                                        scalar2=None, op0=OP.mult)
                t1 = small_p.tile([P, NT], F32, tag="nt_t1", name="nt_t1")[:pp, :n_cols]
                for _ in range(3):
                    nc.vector.tensor_tensor(out=t1, in0=y, in1=y, op=OP.mult)
                    nc.vector.tensor_tensor(out=t1, in0=t1, in1=vh, op=OP.mult)
                    nc.vector.tensor_scalar(out=t1, in0=t1, scalar1=-1.0,
                                            scalar2=1.5, op0=OP.mult, op1=OP.add)
                    nc.vector.tensor_tensor(out=y, in0=y, in1=t1, op=OP.mult)

            def materialize_xT(mv, rs, tiles):
                """ln-apply + transpose x tiles into feature-major xT."""
                xT = xt_p.tile([P, KD, T], MDT, tag="xT", name="xT")
                for t in tiles:
                    lnx = t768_p.tile([P, D], MDT, tag="lnx")
                    nc.gpsimd.tensor_scalar(
                        out=lnx, in0=x_sb[:, t, :], scalar1=mv[:, t, 0:1],
                        scalar2=rs[:, t:t + 1], op0=OP.subtract, op1=OP.mult)
                    for g in range(2):
                        tp = psh_p.tile([P, 3, P], MDT, tag="half", name="tp")
                        for j in range(3):
                            k = g * 3 + j
                            transpose128(tp[:, j, :], lnx[:, k * P:(k + 1) * P],
                                         identm)
                        nc.vector.tensor_copy(
                            out=xT[:, g * 3:(g + 1) * 3, t * P:(t + 1) * P],
                            in_=tp)
                return xT

            def elu1_row(out_ap, src_ap):
                """elu(x)+1 on a [1, D] row."""
                t0 = t768_p.tile([P, D], MDT, tag="el0k", name="t0r")[0:1, :]
                nc.vector.tensor_scalar(out=t0, in0=src_ap, scalar1=0.0,
                                        scalar2=None, op0=OP.min)
                te = t768_p.tile([P, D], MDT, tag="el1k", name="ter")[0:1, :]
                nc.scalar.activation(out=te, in_=t0, func=AF.Exp, bias=0.0, scale=1.0)
                nc.vector.scalar_tensor_tensor(out=out_ap, in0=src_ap, scalar=0.0,
                                               in1=te, op0=OP.max, op1=OP.add)

            def load_w(dram_ap, pool_tag="wproj"):
                """[D, D] dram -> [P, KD, D] sbuf tile."""
                w = wbig_p.tile([P, KD, D], MDT, tag=pool_tag, name="w_" + pool_tag)
                nc.sync.dma_start(out=w, in_=dram_ap.rearrange("(k p) n -> p k n", p=P))
                return w

            def proj_tok(out_ps, lhsT_src, t_or_cols, w_sb, kd=KD):
                """token-major proj: out_ps [P, D] = lhsT_src.T @ W."""
                for k in range(kd):
                    lhsT = lhsT_src(k, t_or_cols)
                    mm(out_ps[:, 0:512], lhsT, w_sb[:, k, 0:512],
                       start=k == 0, stop=k == kd - 1)
                    mm(out_ps[:, 512:D], lhsT, w_sb[:, k, 512:D],
                       start=k == 0, stop=k == kd - 1)

            def resid_stats(t, o_ps, mv):
                nc.vector.tensor_tensor(out=x_sb[:, t, :], in0=x_sb[:, t, :],
                                        in1=o_ps, op=OP.add)
                ln_stats(x_sb[:, t, :], mv[:, t, :])

            # ============ adapter (once; all f32) ============
            with tc.tile_pool(name="wad", bufs=1) as wad_p:
                act_sb = wad_p.tile([P, 3], F32, name="act_sb")
                nc.sync.dma_start(out=act_sb,
                                  in_=act_d.rearrange("(k p) o -> p (k o)", p=P))
                a1_ps = psw_p.tile([P, D], F32, tag="work")
                for k in range(3):
                    kp = P if k < 2 else 64
                    wt = wad_p.tile([P, D], F32, tag="adw1t", bufs=1)
                    nc.sync.dma_start(out=wt[:kp, :], in_=adw1_d[k * P:k * P + kp, :])
                    mm(a1_ps[0:1, 0:512], act_sb[:kp, k:k + 1], wt[:kp, 0:512],
                       start=k == 0, stop=k == 2)
                    mm(a1_ps[0:1, 512:D], act_sb[:kp, k:k + 1], wt[:kp, 512:D],
                       start=k == 0, stop=k == 2)
                a1 = t768_p.tile([P, D], F32, tag="out_sb", bufs=1,
                                 name="a1")[0:1, :]
                nc.vector.tensor_copy(out=a1, in_=a1_ps[0:1, :])
                mv1 = wad_p.tile([1, 2], F32, name="mv1")
                ln_stats(a1, mv1)
                rs1 = wad_p.tile([1, 1], F32, name="rs1")
                newton_rsqrt(rs1, mv1[0:1, 1:2], 1, pp=1)
                nc.vector.tensor_scalar(out=a1, in0=a1, scalar1=mv1[0:1, 0:1],
                                        scalar2=rs1, op0=OP.subtract, op1=OP.mult)
                gl = t768_p.tile([P, D], MDT, tag="lnx", name="gl")[0:1, :]
                nc.scalar.activation(out=gl, in_=a1, func=AF.Gelu, bias=0.0, scale=1.0)
                # transpose row -> column tiles
                a1T = wad_p.tile([P, KD], MDT, name="a1T")
                for k in range(KD):
                    tp = psh_p.tile([P, P], MDT, tag="half", name="tp_ad")
                    transpose128(tp[:, 0:1], gl[0:1, k * P:(k + 1) * P], identm)
                    nc.vector.tensor_copy(out=a1T[:, k:k + 1], in_=tp[:, 0:1])
                # emb = gl @ adw2 (feature-major column)
                embT = const_p.tile([P, KD], MDT, name="embT")
                for m in range(KD):
                    ep = psh_p.tile([P, P], F32, tag="half", name="emb_ps")
                    for k in range(KD):
                        wad2t = wad_p.tile([P, P], MDT, tag="adw2t", name="adw2t")
                        nc.sync.dma_start(
                            out=wad2t,
                            in_=adw2_d[k * P:(k + 1) * P, m * P:(m + 1) * P])
                        mm(ep[:, 0:1], wad2t, a1T[:, k:k + 1],
                           start=k == 0, stop=k == KD - 1)
                    nc.vector.tensor_copy(out=embT[:, m:m + 1], in_=ep[:, 0:1])

            # persistent stat arrays
            mv_a = stat_p.tile([P, NT, 2], F32, name="mv_a")
            rs_a = stat_p.tile([P, NT], F32, name="rs_a")
            mv_b = stat_p.tile([P, NT, 2], F32, name="mv_b")
            rs_b = stat_p.tile([P, NT], F32, name="rs_b")
            mv_c = stat_p.tile([P, NT, 2], F32, name="mv_c")
            rs_c = stat_p.tile([P, NT], F32, name="rs_c")

            def attn_out_phase(xT, wq_sb, bd3, out_w, kvm_sb, mv_next,
                               is_self):
                """q proj (feature-major) + zden + y/c or alpha/o + resid.

                is_self=False: out_w = M_sb [H, D]; out = alpha @ M.
                is_self=True:  out_w = wc_sb [P, KD, D]; y from kvm, c proj.
                """
                for tci in range(NTC):
                    c0 = tci * 512
                    qt = fm_p.tile([P, KD, 512], MDT, tag="fm512", name="qt")
                    for m in range(KD):
                        q_ps = psh_p.tile([P, 512], F32, tag="half", name="q_ps")
                        for k in range(KD):
                            mm(q_ps, wq_sb[:, k, m * P:(m + 1) * P],
                               xT[:, k, c0:c0 + 512],
                               start=k == 0, stop=k == KD - 1)
                        # elu(x)+1 drain
                        t0 = t768_p.tile([P, 512], MDT, tag="el0")
                        nc.vector.tensor_scalar(out=t0, in0=q_ps, scalar1=0.0,
                                                scalar2=None, op0=OP.min)
                        te = t768_p.tile([P, 512], MDT, tag="el1")
                        nc.scalar.activation(out=te, in_=t0, func=AF.Exp,
                                             bias=0.0, scale=1.0)
                        nc.vector.scalar_tensor_tensor(
                            out=qt[:, m, :], in0=q_ps, scalar=0.0,
                            in1=te, op0=OP.max, op1=OP.add)
                    # zden [H, 512]
                    zden_ps = psh_p.tile([H, 512], F32, tag="half", name="zden_ps")
                    for k in range(KD):
                        mm(zden_ps, bd3[:, k, :], qt[:, k, :],
                           start=k == 0, stop=k == KD - 1)
                    if not is_self:
                        # alpha = zden / (zden + eps), rows [H, 512] bf16
                        zt = small_p.tile([H, 512], F32, tag="zrow")
                        nc.vector.tensor_scalar(out=zt, in0=zden_ps, scalar1=1e-6,
                                                scalar2=None, op0=OP.add)
                        rz = small_p.tile([H, 512], F32, tag="zrow", name="rz")
                        nc.vector.reciprocal(out=rz, in_=zt)
                        al = small_p.tile([H, 512], MDT, tag="alpha")
                        nc.vector.tensor_tensor(out=al, in0=zden_ps, in1=rz,
                                                op=OP.mult)
                        for ti in range(4):
                            t = tci * 4 + ti
                            o_ps = psw_p.tile([P, D], F32, tag="work", name="o_ps")
                            mm(o_ps[:, 0:512], al[:, ti * P:(ti + 1) * P],
                               out_w[:, 0:512], start=True, stop=True)
                            mm(o_ps[:, 512:D], al[:, ti * P:(ti + 1) * P],
                               out_w[:, 512:D], start=True, stop=True)
                            resid_stats(t, o_ps, mv_next)
                    else:
                        # z = 1 / (zden + eps) -> bf16 rows
                        zt = small_p.tile([H, 512], F32, tag="zrow")
                        nc.vector.tensor_scalar(out=zt, in0=zden_ps, scalar1=1e-6,
                                                scalar2=None, op0=OP.add)
                        z12 = small_p.tile([H, 512], MDT, tag="z12")
                        with nc.allow_low_precision(reason="z rows bf16"):
                            nc.vector.reciprocal(out=z12, in_=zt)
                        yt = fm_p.tile([P, KD, 512], MDT, tag="fm512", name="yt")
                        for pr in range(NPR):
                            zbc_ps = psh_p.tile([P, 512], F32, tag="half",
                                                name="zbc_ps")
                            mm(zbc_ps, sel[:, pr, :], z12, start=True, stop=True)
                            zbc = small_p.tile([P, 512], MDT, tag="zbc")
                            nc.scalar.activation(out=zbc, in_=zbc_ps, func=AF.Copy,
                                                 bias=0.0, scale=1.0)
                            y_ps = psh_p.tile([P, 512], F32, tag="half", name="y_ps")
                            mm(y_ps[0:64, :], kvm_sb[0:64, pr, 0:64],
                               qt[0:64, pr, :], start=True, stop=True)
                            mm(y_ps[64:P, :], kvm_sb[64:P, pr, 65:129],
                               qt[64:P, pr, :], start=True, stop=True)
                            nc.vector.tensor_tensor(out=yt[:, pr, :], in0=y_ps,
                                                    in1=zbc, op=OP.mult)
                        for ti in range(4):
                            t = tci * 4 + ti
                            c_ps = psw_p.tile([P, D], F32, tag="work", name="c_ps")
                            proj_tok(c_ps,
                                     lambda k, _t: yt[:, k, ti * P:(ti + 1) * P],
                                     t, out_w)
                            resid_stats(t, c_ps, mv_next)

            for rep in range(repeat):
                nc.sync.dma_start(out=x_sb,
                                  in_=x0_d.rearrange("(t p) d -> p t d", p=P))
                for t in range(NT):
                    ln_stats(x_sb[:, t, :], mv_a[:, t, :])

                for l in range(n_layers):
                    # ======== action-attention prep (rows) ========
                    kbd3 = lay_p.tile([P, KD, H], MDT, name="kbd3")
                    nc.vector.memset(kbd3, 0.0)
                    v_bd = lay_p.tile([P, KD, H], MDT, name="v_bd")
                    nc.vector.memset(v_bd, 0.0)
                    M_sb = lay_p.tile([H, D], MDT, name="M_sb")

                    row_ps = psw_p.tile([P, D], F32, tag="work", name="row_ps")
                    for k in range(KD):
                        wt = wbig_p.tile([P, D], MDT, tag="wrow")
                        nc.sync.dma_start(out=wt, in_=ak_d[l, k * P:(k + 1) * P, :])
                        mm(row_ps[0:1, 0:512], embT[:, k:k + 1], wt[:, 0:512],
                           start=k == 0, stop=k == KD - 1)
                        mm(row_ps[0:1, 512:D], embT[:, k:k + 1], wt[:, 512:D],
                           start=k == 0, stop=k == KD - 1)
                    krow = t768_p.tile([P, D], MDT, tag="k_fm",
                                       name="krow")[0:1, :]
                    elu1_row(krow, row_ps[0:1, :])
                    # krow -> block-diag columns kbd3
                    for k in range(KD):
                        tp = psh_p.tile([P, P], MDT, tag="half", name="tp_k")
                        transpose128(tp[:, 0:1], krow[0:1, k * P:(k + 1) * P],
                                     identm)
                        nc.vector.tensor_copy(out=kbd3[0:64, k, 2 * k:2 * k + 1],
                                              in_=tp[0:64, 0:1])
                        nc.vector.tensor_copy(
                            out=kbd3[64:P, k, 2 * k + 1:2 * k + 2],
                            in_=tp[64:P, 0:1])
                    # v row -> block-diag
                    row_ps2 = psw_p.tile([P, D], F32, tag="work", name="row_ps2")
                    for k in range(KD):
                        wt = wbig_p.tile([P, D], MDT, tag="wrow")
                        nc.sync.dma_start(out=wt, in_=av_d[l, k * P:(k + 1) * P, :])
                        mm(row_ps2[0:1, 0:512], embT[:, k:k + 1], wt[:, 0:512],
                           start=k == 0, stop=k == KD - 1)
                        mm(row_ps2[0:1, 512:D], embT[:, k:k + 1], wt[:, 512:D],
                           start=k == 0, stop=k == KD - 1)
                    vrow = t768_p.tile([P, D], MDT, tag="lnx",
                                       name="vrow")[0:1, :]
                    nc.vector.tensor_copy(out=vrow, in_=row_ps2[0:1, :])
                    for k in range(KD):
                        tp = psh_p.tile([P, P], MDT, tag="half", name="tp_v")
                        transpose128(tp[:, 0:1], vrow[0:1, k * P:(k + 1) * P],
                                     identm)
                        nc.vector.tensor_copy(out=v_bd[0:64, k, 2 * k:2 * k + 1],
                                              in_=tp[0:64, 0:1])
                        nc.vector.tensor_copy(
                            out=v_bd[64:P, k, 2 * k + 1:2 * k + 2],
                            in_=tp[64:P, 0:1])
                    # M = v_bd.T @ a_cw  [H, D]
                    m_ps = psw_p.tile([P, D], F32, tag="work", name="m_ps")
                    for k in range(KD):
                        wt = wbig_p.tile([P, D], MDT, tag="wrow")
                        nc.sync.dma_start(out=wt, in_=ac_d[l, k * P:(k + 1) * P, :])
                        mm(m_ps[0:H, 0:512], v_bd[:, k, :], wt[:, 0:512],
                           start=k == 0, stop=k == KD - 1)
                        mm(m_ps[0:H, 512:D], v_bd[:, k, :], wt[:, 512:D],
                           start=k == 0, stop=k == KD - 1)
                    nc.vector.tensor_copy(out=M_sb, in_=m_ps[0:H, :])

                    # ======== a-attn ========
                    newton_rsqrt(rs_a, mv_a[:, :, 1], NT)
                    xT = materialize_xT(mv_a, rs_a, range(NT))
                    wq_a = load_w(aq_d[l])
                    attn_out_phase(xT, wq_a, kbd3, M_sb, None, mv_b,
                                   is_self=False)

                    # ======== s-attn: k/v + kvm ========
                    newton_rsqrt(rs_b, mv_b[:, :, 1], NT)
                    xT = materialize_xT(mv_b, rs_b, range(NT))
                    wk_s = load_w(sk_d[l])
                    wv_s = load_w(sv_d[l])
                    kvm_ps_a = psk_p.tile([P, 3, 130], F32, tag="kvm", name="kvm_a")
                    kvm_ps_b = psk_p.tile([P, 3, 130], F32, tag="kvm", name="kvm_b")

                    def kvm_ps(pr):
                        return (kvm_ps_a if pr < 3 else kvm_ps_b)[:, pr % 3, :]

                    for t in range(NT):
                        k_ps = psw_p.tile([P, D], F32, tag="work", name="k_ps")
                        proj_tok(k_ps,
                                 lambda k, tt: xT[:, k, tt * P:(tt + 1) * P],
                                 t, wk_s)
                        t0 = t768_p.tile([P, D], MDT, tag="el0k")
                        nc.vector.tensor_scalar(out=t0, in0=k_ps, scalar1=0.0,
                                                scalar2=None, op0=OP.min)
                        te = t768_p.tile([P, D], MDT, tag="el1k")
                        nc.scalar.activation(out=te, in_=t0, func=AF.Exp,
                                             bias=0.0, scale=1.0)
                        k_fm = t768_p.tile([P, D], MDT, tag="k_fm")
                        nc.vector.scalar_tensor_tensor(
                            out=k_fm, in0=k_ps, scalar=0.0,
                            in1=te, op0=OP.max, op1=OP.add)
                        v_ps = psw_p.tile([P, D], F32, tag="work", name="v_ps")
                        proj_tok(v_ps,
                                 lambda k, tt: xT[:, k, tt * P:(tt + 1) * P],
                                 t, wv_s)
                        v_aug = t768_p.tile([P, H, 65], MDT, tag="v_aug",
                                            name="v_aug")
                        nc.gpsimd.memset(v_aug[:, :, 64:65], 1.0)
                        nc.scalar.activation(
                            out=v_aug[:, :, 0:64],
                            in_=v_ps.rearrange("p (h e) -> p h e", h=H),
                            func=AF.Copy, bias=0.0, scale=1.0)
                        for pr in range(NPR):
                            mm(kvm_ps(pr), k_fm[:, pr * P:(pr + 1) * P],
                               v_aug[:, 2 * pr:2 * pr + 2, :]
                               .rearrange("p a b -> p (a b)"),
                               start=t == 0, stop=t == NT - 1, skip=True)

                    # ksum block-diag + kvm bf16
                    ksbd3 = lay_p.tile([P, KD, H], MDT, name="ksbd3")
                    nc.vector.memset(ksbd3, 0.0)
                    for k in range(KD):
                        nc.vector.tensor_copy(out=ksbd3[0:64, k, 2 * k:2 * k + 1],
                                              in_=kvm_ps(k)[0:64, 64:65])
                        nc.vector.tensor_copy(
                            out=ksbd3[64:P, k, 2 * k + 1:2 * k + 2],
                            in_=kvm_ps(k)[64:P, 129:130])
                    kvm_sb = lay_p.tile([P, NPR, 130], MDT, name="kvm_sb")
                    nc.scalar.activation(out=kvm_sb[:, 0:3, :], in_=kvm_ps_a,
                                         func=AF.Copy, bias=0.0, scale=1.0)
                    nc.scalar.activation(out=kvm_sb[:, 3:NPR, :], in_=kvm_ps_b,
                                         func=AF.Copy, bias=0.0, scale=1.0)

                    # ======== s-attn: q / z / y / c ========
                    wq_s = load_w(sq_d[l])
                    wc_s = load_w(sc_d[l])
                    attn_out_phase(xT, wq_s, ksbd3, wc_s, kvm_sb, mv_c,
                                   is_self=True)

                    # ======== MLP ========
                    newton_rsqrt(rs_c, mv_c[:, :, 1], NT)
                    xT = materialize_xT(mv_c, rs_c, range(NT))
                    for sp in range(NSP):
                        c0 = sp * SPAN
                        h_sb = h_p.tile([P, NF, SPAN], MDT, tag="h", name="h_sb")
                        for f2 in range(NF // 2):
                            w1t = wmlp_p.tile([P, KD, 256], MDT, tag="w1t")
                            nc.sync.dma_start(
                                out=w1t,
                                in_=w1_d[l, :, f2 * 256:(f2 + 1) * 256]
                                .rearrange("(k p) n -> p k n", p=P))
                            for fj in range(2):
                                fm = f2 * 2 + fj
                                h_ps = psh_p.tile([P, SPAN], F32, tag="half",
                                                  name="h_ps")
                                for k in range(KD):
                                    mm(h_ps, w1t[:, k, fj * P:(fj + 1) * P],
                                       xT[:, k, c0:c0 + SPAN],
                                       start=k == 0, stop=k == KD - 1)
                                nc.scalar.activation(
                                    out=h_sb[:, fm, :], in_=h_ps,
                                    func=AF.Gelu_apprx_tanh, bias=0.0, scale=1.0)
                        o_ps0 = psw_p.tile([P, D], F32, tag="work", name="o_ps0")
                        o_ps1 = psw_p.tile([P, D], F32, tag="work", name="o_ps1")
                        o_both = [o_ps0, o_ps1]
                        for f2 in range(NF // 2):
                            w2t = wmlp_p.tile([P, 2, D], MDT, tag="w2t")
                            nc.sync.dma_start(
                                out=w2t,
                                in_=w2_d[l, f2 * 256:(f2 + 1) * 256, :]
                                .rearrange("(c p) n -> p c n", p=P))
                            for fj in range(2):
                                fc = f2 * 2 + fj
                                for ti in range(2):
                                    mm(o_both[ti][:, 0:512],
                                       h_sb[:, fc, ti * P:(ti + 1) * P],
                                       w2t[:, fj, 0:512],
                                       start=fc == 0, stop=fc == NF - 1,
                                       skip=True)
                                    mm(o_both[ti][:, 512:D],
                                       h_sb[:, fc, ti * P:(ti + 1) * P],
                                       w2t[:, fj, 512:D],
                                       start=fc == 0, stop=fc == NF - 1,
                                       skip=True)
                        for ti in range(2):
                            t = sp * 2 + ti
                            resid_stats(t, o_both[ti], mv_a)

                # ======== final LN + heads ========
                newton_rsqrt(rs_a, mv_a[:, :, 1], NT)
                xT = materialize_xT(mv_a, rs_a, range(16, NT))
                wmu_s = load_w(wmu_d)
                wlv_s = load_w(wlv_d)
                for t in range(16, NT):
                    mu_ps = psw_p.tile([P, D], F32, tag="work", name="mu_ps")
                    proj_tok(mu_ps,
                             lambda k, tt: xT[:, k, tt * P:(tt + 1) * P],
                             t, wmu_s)
                    r0 = (t - 16) * P
                    mu_sb = t768_p.tile([P, D], F32, tag="out_sb", bufs=1,
                                        name="mu_sb")
                    nc.vector.tensor_copy(out=mu_sb, in_=mu_ps)
                    nc.sync.dma_start(out=mu_d[r0:r0 + P, :], in_=mu_sb)
                    lv_ps = psw_p.tile([P, D], F32, tag="work", name="lv_ps")
                    proj_tok(lv_ps,
                             lambda k, tt: xT[:, k, tt * P:(tt + 1) * P],
                             t, wlv_s)
                    lv_sb = t768_p.tile([P, D], F32, tag="out_sb", bufs=1,
                                        name="lv_sb")
                    nc.vector.tensor_scalar(out=lv_sb, in0=lv_ps, scalar1=-10.0,
                                            scalar2=2.0, op0=OP.max, op1=OP.min)
                    nc.sync.dma_start(out=lv_d[r0:r0 + P, :], in_=lv_sb)

    nc.finalize()
    return nc


_NC_CACHE = {}


def _get_nc(dt_mode, repeat):
    key = (dt_mode, repeat)
    if key not in _NC_CACHE:
        _NC_CACHE[key] = build_nc(dt_mode, repeat)
    return _NC_CACHE[key]


def _sel_const():
    mdt = _np_dt(BF16)
    s = np.zeros((H, NPR, P), np.float32)
    for pr in range(NPR):
        s[2 * pr, pr, 0:64] = 1.0
        s[2 * pr + 1, pr, 64:P] = 1.0
    return np.ascontiguousarray(s.astype(mdt))


def make_in_maps(inputs, dt_mode=DT_MODE):
    """Shard full inputs -> per-core input dicts."""
    mdt = _np_dt(BF16)
    ctx = np.asarray(inputs['context_latents'], np.float32)     # [8, CL, D]
    acts = np.asarray(inputs['action_latents'], np.float32)     # [8, 320]
    idx = np.asarray(inputs['target_indices'])                  # [8, TT]
    mq = np.asarray(inputs['mq'], np.float32)                   # [G, D]

    adw1 = np.zeros((A_PAD, D), np.float32)
    adw1[:320, :] = np.asarray(inputs['ad_w1'], np.float32)

    def cvt(name):
        return np.ascontiguousarray(np.asarray(inputs[name]).astype(mdt))

    shared = {
        'adw1': adw1,
        'adw2': cvt('ad_w2'),
        'aq': cvt('a_qw'), 'ak': cvt('a_kw'), 'av': cvt('a_vw'), 'ac': cvt('a_cw'),
        'sq': cvt('s_qw'), 'sk': cvt('s_kw'), 'sv': cvt('s_vw'), 'sc': cvt('s_cw'),
        'w1': cvt('mlp_w1'), 'w2': cvt('mlp_w2'),
        'wmu': cvt('mu_w'), 'wlv': cvt('lv_w'),
        'sel': _sel_const(),
    }
    in_maps = []
    for b in range(8):
        queries = mq[idx[b]]                                    # [TT, D]
        x0 = np.concatenate([ctx[b], queries], axis=0)          # [T, D]
        a = np.zeros((A_PAD, 1), np.float32)
        a[:320, 0] = acts[b]
        in_maps.append({'x0': np.ascontiguousarray(x0), 'act': a, **shared})
    return in_maps


def kernel(**inputs):
    nc = _get_nc(DT_MODE, REPEAT)
    in_maps = make_in_maps(inputs, DT_MODE)
    r = run_bass_kernel_spmd(nc, in_maps, list(range(8)))
    mu = np.stack([r.results[b]['mu'] for b in range(8)])
    lv = np.stack([r.results[b]['lv'] for b in range(8)])
    return mu, lv
